# revision 1
# baseline (speedup 1.0000x reference)
"""Trainium2 Bass kernel for the 2-layer LSTM decoder (batch=1, T=512 autoregressive).

Strategy: the 512 decode steps are strictly sequential (h_t depends on h_{t-1}),
and in this toolchain the ncfw collectives cost ~0.5 ms per call and the
SBUF<->SBUF remote-DMA path does not compile, so cross-core exchange every step
is a net loss.  Instead every core runs the full recurrence independently
(SPMD-replicated); weights are stored fp16 and streamed HBM->SBUF each step
(96 MB/step) through a 6-deep ring of SBUF chunk buffers, overlapped with the
PE matvecs.  Gates accumulate in PSUM (fp32); activations on the Scalar engine;
elementwise c/h updates on the Vector engine; per-step output row DMA'd to DRAM
with a register-indexed access pattern.

Gate layout: rows are reordered host-side to [i, f, o, g] so sigmoid covers
PSUM columns 0:48 and tanh covers 48:64 in single activation instructions.
"""

import numpy as np

E = 2048
D = 128
T = 512
N_CORES = 8
NSLOT = 4  # streamed-chunk ring depth (4 MB col-major chunks)

_CACHE = {}


def _build(n_steps):
    import concourse.bass as bass
    from concourse import mybir

    f16 = mybir.dt.float16
    f32 = mybir.dt.float32
    AF = mybir.ActivationFunctionType
    OP = mybir.AluOpType

    nc = bass.Bass(num_devices=N_CORES)

    whh0T = nc.declare_dram_parameter("whh0T", [1024, 16384], f16, isOutput=False)
    whh1T = nc.declare_dram_parameter("whh1T", [1024, 16384], f16, isOutput=False)
    wih1T = nc.declare_dram_parameter("wih1T", [1024, 16384], f16, isOutput=False)
    wih0T_d = nc.declare_dram_parameter("wih0T", [D, 4 * E], f16, isOutput=False)
    woutT_d = nc.declare_dram_parameter("woutT", [128, 16 * 128], f16, isOutput=False)
    b0_d = nc.declare_dram_parameter("b0c", [128, 64], f32, isOutput=False)
    b1_d = nc.declare_dram_parameter("b1c", [128, 64], f32, isOutput=False)
    bout_d = nc.declare_dram_parameter("boutc", [128, 1], f32, isOutput=False)
    lat_d = nc.declare_dram_parameter("lat16", [128, 16], f16, isOutput=False)
    x0_d = nc.declare_dram_parameter("x016", [128, 1], f16, isOutput=False)
    zero_d = nc.declare_dram_parameter("zeros", [128, 16], f32, isOutput=False)
    y = nc.declare_dram_parameter("y", [n_steps, 128], f32, isOutput=True)

    import contextlib
    with contextlib.ExitStack() as ctx:
        en = ctx.enter_context

        # SBUF buffers
        slots = [en(nc.sbuf_tensor(f"slot{i}", [128, 16384], f16)) for i in range(NSLOT)]
        wih0 = en(nc.sbuf_tensor("wih0", [128, 4 * E], f16))
        wout = en(nc.sbuf_tensor("wout", [128, 16 * 128], f16))
        b0 = en(nc.sbuf_tensor("b0", [128, 64], f32))
        b1 = en(nc.sbuf_tensor("b1", [128, 64], f32))
        bout = en(nc.sbuf_tensor("bout", [128, 1], f32))
        h0 = en(nc.sbuf_tensor("h0", [128, 16], f16))
        h1 = en(nc.sbuf_tensor("h1", [128, 16], f16))
        x16 = en(nc.sbuf_tensor("x16", [128, 1], f16))
        x_sb = en(nc.sbuf_tensor("x_sb", [128, 1], f32))
        c0 = en(nc.sbuf_tensor("c0", [128, 16], f32))
        c1 = en(nc.sbuf_tensor("c1", [128, 16], f32))
        gsb = en(nc.sbuf_tensor("gsb", [128, 64], f32))
        si = en(nc.sbuf_tensor("si", [128, 48], f32))
        tg = en(nc.sbuf_tensor("tg", [128, 16], f32))
        t1 = en(nc.sbuf_tensor("t1", [128, 16], f32))
        t2 = en(nc.sbuf_tensor("t2", [128, 16], f32))
        tc = en(nc.sbuf_tensor("tc", [128, 16], f32))

        # PSUM
        g0 = en(nc.psum_tensor("g0", [128, 64], f32))
        g1 = en(nc.psum_tensor("g1", [128, 64], f32))
        px = en(nc.psum_tensor("px", [128, 1], f32))

        # semaphores
        slotsem = [en(nc.semaphore(f"slotsem{i}")) for i in range(NSLOT)]  # per-slot chunk arrivals
        pe_done = en(nc.semaphore("pe_done"))    # PE released a slot (+1)
        pe2post = en(nc.semaphore("pe2post"))    # PE -> DVE: psum group done (+1) x3/step
        dve2act = en(nc.semaphore("dve2act"))    # DVE -> ACT (+1) x4/step
        act2dve = en(nc.semaphore("act2dve"))    # ACT -> DVE (+1) x4/step
        post2pe = en(nc.semaphore("post2pe"))    # DVE -> PE: h0/h1/x16 ready (+1) x3/step
        dve2g = en(nc.semaphore("dve2g"))        # DVE -> gpsimd: x_sb ready (+1)
        odma = en(nc.semaphore("odma"))          # output row DMA done (+16)
        init_sem = en(nc.semaphore("init_sem"))

        # registers
        pthr = en(nc.sync.register("pthr"))     # sync: pe_done threshold
        sthr = [en(nc.tensor.register(f"sthr{i}")) for i in range(NSLOT)]  # PE: per-slot thresholds
        p2thr = en(nc.tensor.register("p2thr"))  # PE: post2pe threshold
        vpthr = en(nc.vector.register("vpthr"))  # DVE: pe2post threshold
        vathr = en(nc.vector.register("vathr"))  # DVE: act2dve threshold
        vothr = en(nc.vector.register("vothr"))  # DVE: odma threshold
        sathr = en(nc.scalar.register("sathr"))  # ACT: dve2act threshold
        gthr = en(nc.gpsimd.register("gthr"))   # gpsimd: dve2g threshold
        trow = en(nc.gpsimd.register("trow"))   # gpsimd: output row index

        with nc.Block() as blk:
            @blk.gpsimd
            def _(g):
                g.reg_mov(gthr, 0)
                g.reg_mov(trow, 0)
                for i, (dst, src) in enumerate([
                    (h0[:, :], lat_d[:, :]),
                    (h1[:, :], lat_d[:, :]),
                    (x16[:, :], x0_d[:, :]),
                    (wih0[:, :], wih0T_d[:, :]),
                    (wout[:, :], woutT_d[:, :]),
                    (b0[:, :], b0_d[:, :]),
                    (b1[:, :], b1_d[:, :]),
                    (bout[:, :], bout_d[:, :]),
                    (c0[:, :], zero_d[:, :]),
                    (c1[:, :], zero_d[:, :]),
                ]):
                    g.dma_start(out=dst, in_=src).then_inc(init_sem, 16)
                g.wait_ge(init_sem, 16 * 10)
                g.sem_inc(pe_done, NSLOT)   # ring starts empty
                g.sem_inc(post2pe, 1)       # x16 (=x0) is ready for iter 0
                g.sem_inc(odma, 16)         # virtual "previous" output DMA
                g.sem_inc(init_sem, 1)      # release other engines

            @blk.sync
            def _(s):
                s.reg_mov(pthr, 0)
                s.wait_ge(init_sem, 16 * 10 + 1)

            @blk.tensor
            def _(t):
                for r in sthr:
                    t.reg_mov(r, 0)
                t.reg_mov(p2thr, 0)
                t.wait_ge(init_sem, 16 * 10 + 1)

            @blk.vector
            def _(v):
                v.reg_mov(vpthr, 0)
                v.reg_mov(vathr, 0)
                v.reg_mov(vothr, 0)
                v.wait_ge(init_sem, 16 * 10 + 1)

            @blk.scalar
            def _(sc):
                sc.reg_mov(sathr, 0)
                sc.wait_ge(init_sem, 16 * 10 + 1)

        # streamed chunk order: u in [0,8): whh0 group u ; u in [8,24): pairs
        # (whh1 c, wih1 c) with c=(u-8)//2, even u -> whh1, odd -> wih1
        def chunk_src(u):
            if u < 8:
                return whh0T[128 * u : 128 * (u + 1), :]
            c = (u - 8) // 2
            mat = whh1T if (u - 8) % 2 == 0 else wih1T
            return mat[128 * c : 128 * (c + 1), :]
        N_CHUNK = 24

        with nc.Fori(0, n_steps):
            # ---------------- sync engine: stream 48 weight chunks ----------------
            for u in range(N_CHUNK):
                nc.sync.reg_add(pthr, pthr, 1)
                nc.sync.wait_ge(pe_done, nc.sync.snap(pthr))
                nc.sync.dma_start(out=slots[u % NSLOT][:, :], in_=chunk_src(u)).then_inc(slotsem[u % NSLOT], 16)

            # ---------------- PE ----------------
            # wait x16 ready (also guarantees g0/g1 psum free)
            nc.tensor.reg_add(p2thr, p2thr, 1)
            nc.tensor.wait_ge(post2pe, nc.tensor.snap(p2thr))

            def wait_slot(u):
                s = u % NSLOT
                nc.tensor.reg_add(sthr[s], sthr[s], 16)
                nc.tensor.wait_ge(slotsem[s], nc.tensor.snap(sthr[s]))

            # L0: for each col-group c, per column: whh0 k=0..15 then x-part
            for c in range(8):
                wait_slot(c)
                sl = slots[c % NSLOT]
                for ml in range(8):
                    j = 8 * c + ml
                    for k in range(16):
                        nc.tensor.matmul(g0[:, j : j + 1],
                                         sl[:, 1024 * k + 128 * ml : 1024 * k + 128 * (ml + 1)],
                                         h0[:, k : k + 1], start=(k == 0), stop=False)
                    mm = nc.tensor.matmul(g0[:, j : j + 1],
                                          wih0[:, 128 * j : 128 * (j + 1)],
                                          x16[:, :], start=False, stop=True)
                if c == 7:
                    nc.tensor.drain()
                    nc.tensor.sem_inc(pe2post, 1)
                    nc.tensor.sem_inc(pe_done, 1)
                else:
                    mm.then_inc(pe_done, 1)
            # L1: wait h0 of this step, then per col-group pair (whh1 c, wih1 c)
            nc.tensor.reg_add(p2thr, p2thr, 1)
            nc.tensor.wait_ge(post2pe, nc.tensor.snap(p2thr))
            for c in range(8):
                u_hh = 8 + 2 * c
                u_ih = 9 + 2 * c
                wait_slot(u_hh)
                wait_slot(u_ih)
                sl_hh = slots[u_hh % NSLOT]
                sl_ih = slots[u_ih % NSLOT]
                for ml in range(8):
                    j = 8 * c + ml
                    for k in range(16):
                        nc.tensor.matmul(g1[:, j : j + 1],
                                         sl_hh[:, 1024 * k + 128 * ml : 1024 * k + 128 * (ml + 1)],
                                         h1[:, k : k + 1], start=(k == 0), stop=False)
                    for k in range(16):
                        mm = nc.tensor.matmul(g1[:, j : j + 1],
                                              sl_ih[:, 1024 * k + 128 * ml : 1024 * k + 128 * (ml + 1)],
                                              h0[:, k : k + 1], start=False, stop=(k == 15))
                if c == 7:
                    nc.tensor.drain()
                    nc.tensor.sem_inc(pe2post, 1)
                    nc.tensor.sem_inc(pe_done, 1)
                else:
                    mm.then_inc(pe_done, 1)
                nc.tensor.sem_inc(pe_done, 1)
            # W_out @ h1 (needs h1 of this step)
            nc.tensor.reg_add(p2thr, p2thr, 1)
            nc.tensor.wait_ge(post2pe, nc.tensor.snap(p2thr))
            for k in range(16):
                mm = nc.tensor.matmul(px[:, :], wout[:, 128 * k : 128 * (k + 1)],
                                      h1[:, k : k + 1], start=(k == 0), stop=(k == 15))
            nc.tensor.drain()
            nc.tensor.sem_inc(pe2post, 1)

            # ---------------- DVE (vector) ----------------
            for cell, (gps, bsb, csb, hsb) in enumerate([(g0, b0, c0, h0), (g1, b1, c1, h1)]):
                nc.vector.reg_add(vpthr, vpthr, 1)
                nc.vector.wait_ge(pe2post, nc.vector.snap(vpthr))          # psum group ready
                nc.vector.tensor_tensor(out=gsb[:, :], in0=gps[:, :], in1=bsb[:, :],
                                        op=OP.add)
                nc.vector.drain()
                nc.vector.sem_inc(dve2act, 1)
                nc.vector.reg_add(vathr, vathr, 1)
                nc.vector.wait_ge(act2dve, nc.vector.snap(vathr))          # sig/tanh ready
                nc.vector.tensor_tensor(out=t1[:, :], in0=si[:, 0:16], in1=tg[:, :], op=OP.mult)
                nc.vector.tensor_tensor(out=t2[:, :], in0=si[:, 16:32], in1=csb[:, :], op=OP.mult)
                nc.vector.drain()
                nc.vector.tensor_tensor(out=csb[:, :], in0=t1[:, :], in1=t2[:, :],
                                        op=OP.add)
                nc.vector.drain()
                nc.vector.sem_inc(dve2act, 1)
                nc.vector.reg_add(vathr, vathr, 1)
                nc.vector.wait_ge(act2dve, nc.vector.snap(vathr))          # tanh(c) ready
                nc.vector.tensor_tensor(out=hsb[:, :], in0=si[:, 32:48], in1=tc[:, :],
                                        op=OP.mult)
                nc.vector.drain()
                nc.vector.sem_inc(post2pe, 1)
            # x = px + bout ; x16 = fp16(x)
            nc.vector.reg_add(vpthr, vpthr, 1)
            nc.vector.wait_ge(pe2post, nc.vector.snap(vpthr))              # px ready
            nc.vector.reg_add(vothr, vothr, 16)
            nc.vector.wait_ge(odma, nc.vector.snap(vothr))                 # previous out row flushed
            nc.vector.tensor_tensor(out=x_sb[:, :], in0=px[:, :], in1=bout[:, :], op=OP.add)
            nc.vector.drain()
            nc.vector.tensor_copy(out=x16[:, :], in_=x_sb[:, :])
            nc.vector.drain()
            nc.vector.sem_inc(dve2g, 1)
            nc.vector.sem_inc(post2pe, 1)

            # ---------------- ACT (scalar) ----------------
            for cell, csb in enumerate([c0, c1]):
                nc.scalar.reg_add(sathr, sathr, 1)
                nc.scalar.wait_ge(dve2act, nc.scalar.snap(sathr))
                nc.scalar.activation(si[:, :], gsb[:, 0:48], AF.Sigmoid)
                nc.scalar.activation(tg[:, :], gsb[:, 48:64], AF.Tanh)
                nc.scalar.drain()
                nc.scalar.sem_inc(act2dve, 1)
                nc.scalar.reg_add(sathr, sathr, 1)
                nc.scalar.wait_ge(dve2act, nc.scalar.snap(sathr))
                nc.scalar.activation(tc[:, :], csb[:, :], AF.Tanh)
                nc.scalar.drain()
                nc.scalar.sem_inc(act2dve, 1)

            # ---------------- gpsimd: output row ----------------
            nc.gpsimd.reg_add(gthr, gthr, 1)
            nc.gpsimd.wait_ge(dve2g, nc.gpsimd.snap(gthr))
            rv = nc.gpsimd.snap(trow)
            nc.gpsimd.dma_start(out=y[bass.ds(rv, 1), :], in_=x_sb[:, :]).then_inc(odma, 16)
            nc.gpsimd.reg_add(trow, trow, 1)

        with nc.Block() as blk2:
            @blk2.gpsimd
            def _(g):
                g.wait_ge(odma, 16 * (n_steps + 1))

    return nc


def _prep_inputs(latent_space, W_ih0, W_hh0, b_ih0, b_hh0,
                 W_ih1, W_hh1, b_ih1, b_hh1, W_out, b_out):
    f16 = np.float16

    def reorder(W):
        return np.concatenate([W[0:E], W[E:2 * E], W[3 * E:4 * E], W[2 * E:3 * E]], axis=0)

    def bias_lay(b_ih, b_hh):
        br = reorder((b_ih + b_hh).astype(np.float32).reshape(4 * E, 1))[:, 0]
        return np.ascontiguousarray(br.reshape(64, 128).T)

    lat = np.asarray(latent_space, np.float32)[0]
    x0 = np.asarray(W_out, np.float32) @ lat + np.asarray(b_out, np.float32)

    def colmajor(WT):
        out = np.empty((1024, 16384), f16)
        for c in range(8):
            for k in range(16):
                out[128 * c:128 * (c + 1), 1024 * k:1024 * (k + 1)] = \
                    WT[128 * k:128 * (k + 1), 1024 * c:1024 * (c + 1)]
        return out

    wih0T = np.ascontiguousarray(reorder(np.asarray(W_ih0)).T.astype(f16))      # [128, 8192]
    whh0T = colmajor(reorder(np.asarray(W_hh0)).T.astype(f16))
    wih1T = colmajor(reorder(np.asarray(W_ih1)).T.astype(f16))
    whh1T = colmajor(reorder(np.asarray(W_hh1)).T.astype(f16))
    WoT = np.asarray(W_out, np.float32).T.astype(f16)                            # [2048, 128]
    woutT = np.ascontiguousarray(np.hstack([WoT[128 * k:128 * (k + 1), :] for k in range(16)]))

    ins = {
        "whh0T": whh0T, "whh1T": whh1T, "wih1T": wih1T, "wih0T": wih0T,
        "woutT": woutT,
        "b0c": np.ascontiguousarray(bias_lay(b_ih0, b_hh0)),
        "b1c": np.ascontiguousarray(bias_lay(b_ih1, b_hh1)),
        "boutc": np.asarray(b_out, np.float32).reshape(128, 1).copy(),
        "lat16": np.ascontiguousarray(lat.reshape(16, 128).T.astype(f16)),
        "zeros": np.zeros((128, 16), np.float32),
        "x016": np.ascontiguousarray(x0.reshape(128, 1).astype(f16)),
    }
    return ins, x0


def kernel(latent_space, W_ih0, W_hh0, b_ih0, b_hh0,
           W_ih1, W_hh1, b_ih1, b_hh1, W_out, b_out, number_outputs):
    from concourse.bass_utils import run_bass_kernel_spmd

    n_out = int(number_outputs)
    n_steps = n_out - 1

    ins, x0 = _prep_inputs(latent_space, W_ih0, W_hh0, b_ih0, b_hh0,
                           W_ih1, W_hh1, b_ih1, b_hh1, W_out, b_out)

    if n_steps not in _CACHE:
        _CACHE[n_steps] = _build(n_steps)
    nc = _CACHE[n_steps]

    in_maps = [ins for _ in range(N_CORES)]
    res = run_bass_kernel_spmd(nc, in_maps, core_ids=list(range(N_CORES)))
    rows = res.results[0]["y"]                     # [n_steps, 128] fp32
    out = np.concatenate([x0[None, :].astype(np.float32), rows], axis=0)
    return out[None]                               # (1, T, 128) float32

